# revision 14
# baseline (speedup 1.0000x reference)
"""Bass/Trainium2 kernel for a 2-layer LSTM (T=256, B=64, H=IN=1024) on 8 cores.

Sharding: tensor-parallel over the gate/hidden dimension. Core c owns the
H-slice [128c, 128c+128) of every gate (local gate column order i,f,o,g =
512 columns) and therefore the H-slice [128c, 128c+128) of h and c. The full
batch B=64 lives on every core.

Per step and layer, gates[B,512] = x_t @ W_ih^T + h_{t-1} @ W_hh^T + b are
computed activation-stationary: stationary = transposed activation tiles
[K=128, M<=128], moving = pre-transposed weight columns [K=128, N=512]
(fp32r, 1 col/cycle). The input projection is batched two steps at a time
(stationary [128,128] = 2 steps x 64 batch) at full PE width; the recurrent
part is per-step [128,64].

h_t is produced [B=64, 128], transposed on the tensor engine to [128, 64],
and all-gathered every step via remote_dma_broadcast using an XOR slot
scheme: receiver's slot j holds the tile of core (c XOR j); each core's
W_hh^T/W_ih2^T K-chunks are permuted on the host in the same XOR order, so
the program is identical on all cores. Cross-core arrival/release waits are
injected into the scheduled program after TileContext exits (Tile's
single-core scheduling simulator cannot model cross-core semaphores).
"""
import sys

sys.path.insert(0, "/opt/trn_rl_repo")

import numpy as np
import concourse.bass as bass
import concourse.bacc as bacc
import concourse.tile as tile
import concourse.mybir as mybir
from concourse.bass_utils import run_bass_kernel_spmd

N = 8           # cores
B = 64          # batch
H = 1024        # hidden
IN = 1024       # input size
L = 2           # layers
GL = 512        # local gate columns (4 gates x 128)
HL = 128        # local hidden slice
F32 = mybir.dt.float32
F32R = mybir.dt.float32r

# last run's profiling info (filled when trace=True)
LAST_EXEC_NS = None


def _inject_waits(nc, pending):
    """pending: list of (anchor_inst_name, engine_type, sem, val). Insert a bare
    EventSemaphore wait immediately before each anchor in its scheduled block."""
    f = nc.m.functions[0]
    loc = {}
    for b in f.blocks:
        for idx, i in enumerate(b.instructions):
            loc[i.name] = (b, idx)
    per_block = {}
    for name, eng, sem, val in pending:
        blk, idx = loc[name]
        per_block.setdefault(blk.name, (blk, []))[1].append((idx, eng, sem, val))
    for _, (blk, items) in per_block.items():
        il = blk.instructions
        for idx, eng, sem, val in sorted(items, key=lambda x: -x[0]):
            w = nc.engines[eng].wait_ge(sem, val)
            tail = nc.cur_bb.bb.instructions
            assert tail[-1].name == w.ins.name
            tail.pop()
            il.insert(idx, w.ins)


def build(T):
    """Build + compile the SPMD bass module for sequence length T (even)."""
    import os
    NO_REMOTE = bool(int(os.environ.get("NO_REMOTE", "0")))
    NO_WAITS = bool(int(os.environ.get("NO_WAITS", "0"))) or NO_REMOTE
    assert T % 2 == 0
    TB = T * B
    n_pairs = T // 2
    R = 4  # AG pair-buffer rotation depth (pairs)

    nc = bacc.Bacc(None, target_bir_lowering=False, debug=False)

    xT = nc.dram_tensor("xT", [IN, TB], F32R, kind="ExternalInput")
    wih = [nc.dram_tensor(f"wih{l}", [IN, GL], F32R, kind="ExternalInput") for l in range(L)]
    whh = [nc.dram_tensor(f"whh{l}", [H, GL], F32R, kind="ExternalInput") for l in range(L)]
    bias_d = nc.dram_tensor("bias", [L, GL], F32R, kind="ExternalInput")
    h0ag_d = nc.dram_tensor("h0ag", [L, 128, N * B], F32R, kind="ExternalInput")
    c0_d = nc.dram_tensor("c0loc", [L, B, HL], F32, kind="ExternalInput")
    ident_d = nc.dram_tensor("ident", [B, B], F32, kind="ExternalInput")
    ones_d = nc.dram_tensor("ones", [1, 128], F32R, kind="ExternalInput")

    y_d = nc.dram_tensor("y", [T, B, HL], F32, kind="ExternalOutput")
    hn_d = nc.dram_tensor("hn", [L, B, HL], F32, kind="ExternalOutput")
    cn_d = nc.dram_tensor("cn", [L, B, HL], F32, kind="ExternalOutput")

    rs = [nc.alloc_semaphore("rs1"), nc.alloc_semaphore("rs2")]
    ls = nc.alloc_semaphore("lsend")
    pending = []  # (inst_name, engine, sem, val, cls)
    pool_insts = {}  # inst_name -> ("prep"|"trig", l, t)

    with tile.TileContext(nc) as tc:
        with (
            tc.tile_pool(name="wp", bufs=1) as wp,
            tc.tile_pool(name="xtp", bufs=3) as xtp,
            tc.tile_pool(name="agp0", bufs=R) as agp0,
            tc.tile_pool(name="agp1", bufs=R) as agp1,
            tc.tile_pool(name="ewp", bufs=3) as ewp,
            tc.tile_pool(name="hp", bufs=4) as hp,
            tc.tile_pool(name="cp", bufs=2) as cp,
            tc.tile_pool(name="psp", bufs=2, space="PSUM") as psp,
        ):
            # ---- persistent tiles ----
            wih_sb, whh_sb = [], []
            for l in range(L):
                wi = wp.tile([128, 8 * GL], F32R, tag=f"wih{l}")
                wh = wp.tile([128, 8 * GL], F32R, tag=f"whh{l}")
                for k in range(8):
                    nc.sync.dma_start(wi[:, k * GL:(k + 1) * GL], wih[l][k * 128:(k + 1) * 128, :])
                    nc.sync.dma_start(wh[:, k * GL:(k + 1) * GL], whh[l][k * 128:(k + 1) * 128, :])
                wih_sb.append(wi)
                whh_sb.append(wh)
            bias_sb = []
            for l in range(L):
                bt = wp.tile([1, GL], F32R, tag=f"bias{l}", name=f"bias{l}")
                nc.sync.dma_start(bt[:], bias_d[l:l + 1, :].opt())
                bias_sb.append(bt)
            ident_sb = wp.tile([B, B], F32, tag="ident")
            nc.sync.dma_start(ident_sb[:], ident_d[:])
            ones_sb = wp.tile([1, 128], F32R, tag="ones")
            nc.sync.dma_start(ones_sb[:], ones_d[:])
            h0_sb = []
            for l in range(L):
                t0 = wp.tile([128, N * B], F32R, tag=f"h0ag{l}")
                nc.sync.dma_start(t0[:], h0ag_d[l])
                h0_sb.append(t0)
            c_prev = []
            for l in range(L):
                ct = cp.tile([B, HL], F32, tag=f"c{l}")
                nc.sync.dma_start(ct[:], c0_d[l])
                c_prev.append(ct)

            ag_tiles = [dict(), dict()]   # layer -> pair -> AG tile [128, 8*128]
            gates_tile = [None, None]
            agpools = [agp0, agp1]

            def step(l, t):
                p, q = t >> 1, t & 1
                if q == 0:
                    # ---- pair start: AG tile + x-part + bias matmuls ----
                    ag_tiles[l][p] = agpools[l].tile(
                        [128, 64 + N * 128], F32R, tag=f"ag{l}", name=f"ag{l}_{p}")
                    if p - R in ag_tiles[l]:
                        del ag_tiles[l][p - R]
                    g = psp.tile([128, GL], F32, tag=f"g{l}")
                    gates_tile[l] = g
                    if l == 0:
                        xt_t = xtp.tile([128, 8 * 128], F32R, tag="xt")
                        for k in range(8):
                            nc.sync.dma_start(
                                xt_t[:, k * 128:(k + 1) * 128],
                                xT[k * 128:(k + 1) * 128, p * 128:(p + 1) * 128])
                        stat = xt_t
                        rhsW = wih_sb[0]
                    else:
                        stat = ag_tiles[0][p]
                        rhsW = wih_sb[1]
                    off = 0 if l == 0 else 64
                    for k in range(8):
                        mm = nc.tensor.matmul(
                            g[:], stat[:, off + k * 128: off + (k + 1) * 128],
                            rhsW[:, k * GL:(k + 1) * GL],
                            start=(k == 0), stop=False)
                        if l == 1 and k == 0:
                            pending.append((mm.ins.name, mm.ins.engine, rs[0], 14 * (2 * p + 2), 'arrival'))
                    nc.tensor.matmul(g[:], ones_sb[:], bias_sb[l][:],
                                     start=False, stop=False)
                g = gates_tile[l]
                # ---- recurrent part for step t ----
                if t == 0:
                    prev = h0_sb[l]
                    pslice = lambda j: prev[:, j * B:(j + 1) * B]
                    out_ap, m = g[0:B, :], B
                elif q == 0:
                    # h_{t-1} is the parity-1 tile of the previous pair
                    prev = ag_tiles[l][(t - 1) >> 1]
                    pslice = lambda j: prev[:, 64 + j * 128 + B: 64 + (j + 1) * 128]
                    out_ap, m = g[0:B, :], B
                else:
                    # h_{t-1} is the parity-0 tile of this pair; use a 128-col
                    # window [garbage | h_{t-1}] so the contribution lands in
                    # rows 64:128 (rows 0:64 already consumed -> garbage ok)
                    prev = ag_tiles[l][p]
                    pslice = lambda j: prev[:, j * 128: (j + 1) * 128]
                    out_ap, m = g[:, :], 128
                for j in range(8):
                    mm = nc.tensor.matmul(
                        out_ap, pslice(j),
                        whh_sb[l][:, j * GL:(j + 1) * GL],
                        start=False, stop=(j == 7))
                    if j == 0 and t >= 1:
                        pending.append((mm.ins.name, mm.ins.engine, rs[l], 14 * t, 'arrival'))
                # ---- elementwise ----
                gates = g[q * B:(q + 1) * B, :]
                sig = ewp.tile([B, 384], F32, tag="sig")
                nc.scalar.activation(sig[:], gates[:, 0:384],
                                     mybir.ActivationFunctionType.Sigmoid)
                tg = ewp.tile([B, HL], F32, tag="tg")
                nc.scalar.activation(tg[:], gates[:, 384:512],
                                     mybir.ActivationFunctionType.Tanh)
                t1 = ewp.tile([B, HL], F32, tag="t1")
                nc.vector.tensor_mul(t1[:], sig[:, 128:256], c_prev[l][:])
                t2 = ewp.tile([B, HL], F32, tag="t2")
                nc.vector.tensor_mul(t2[:], sig[:, 0:128], tg[:])
                cn_t = cp.tile([B, HL], F32, tag=f"c{l}")
                nc.vector.tensor_add(cn_t[:], t1[:], t2[:])
                c_prev[l] = cn_t
                tc2 = ewp.tile([B, HL], F32, tag="tc")
                nc.scalar.activation(tc2[:], cn_t[:], mybir.ActivationFunctionType.Tanh)
                hn_t = hp.tile([B, HL], F32, tag=f"h{l}")
                nc.vector.tensor_mul(hn_t[:], sig[:, 256:384], tc2[:])
                # ---- transpose to [128, 64] and broadcast ----
                if not (l == 1 and t == T - 1):
                    tp = psp.tile([128, B], F32, tag="tp")
                    nc.tensor.transpose(tp[:], hn_t[:], ident_sb[:])
                    dst = ag_tiles[l][p][:, 64 + q * B: 64 + q * B + B]
                    nc.vector.tensor_copy(dst, tp[:])
                    for d in range(1, N) if not NO_REMOTE else []:
                        rdests = [None] * 8
                        rdests[d] = (0, d ^ 2 if d >= 4 else d)
                        prep = nc.gpsimd.remote_dma_broadcast(
                            out_ap=ag_tiles[l][p][:, 64 + d * 128 + q * B: 64 + d * 128 + q * B + B],
                            in_ap=dst, remote_sem=rs[l], local_sem=ls, rdests=rdests)
                        pool_insts[prep.ins.name] = ("prep", l, t)
                        if d == 1 and l == 0 and t >= 9:
                            # SWDGE ring depth limit: sends through step t-5
                            # (both layers) must have drained
                            pending.append((prep.ins.name, prep.ins.engine, ls, 224 * (t - 4), 'ring'))
                    trig = nc.gpsimd.trigger_dma(count=None) if not NO_REMOTE else None
                    if trig is not None:
                        pool_insts[trig.ins.name] = ("trig", l, t)
                    # buffer-release waits: peers must be done reading the AG
                    # buffer this send overwrites (pair p-R), proven via the
                    # data semaphores themselves.
                    if p >= R and not NO_REMOTE:
                        v1 = 14 * (2 * (p - R) + 3)
                        v2 = 14 * (2 * (p - R) + 1)
                        if l == 0:
                            pending.append((trig.ins.name, trig.ins.engine, rs[0], v1, 'release'))
                            pending.append((trig.ins.name, trig.ins.engine, rs[1], v2, 'release'))
                        else:
                            pending.append((trig.ins.name, trig.ins.engine, rs[1], v1, 'release'))
                # ---- outputs ----
                if l == 1:
                    nc.sync.dma_start(y_d[t], hn_t[:])
                if t == T - 1:
                    nc.sync.dma_start(hn_d[l], hn_t[:])
                    st = nc.sync.dma_start(cn_d[l], cn_t[:])
                    if l == 1:
                        # final drain: consume every remote increment so the
                        # semaphore state is deterministic for re-execution;
                        # anchored here because this DMA is forced last by data
                        # deps (a bare nop would float to the stream front)
                        pending.append((st.ins.name, st.ins.engine, rs[0], 14 * T, 'final'))
                        pending.append((st.ins.name, st.ins.engine, rs[1], 14 * (T - 1), 'final'))
                        pending.append((st.ins.name, st.ins.engine, ls, 112 * (2 * T - 1), 'final_ls'))

            for w in range(T + 2):
                if w < T:
                    step(0, w)
                if w >= 2:
                    step(1, w - 2)



    wait_classes = set(os.environ.get("WAIT_CLASSES", "arrival,release,final").split(","))
    if NO_WAITS:
        wait_classes = set()
    pending = [pw[:4] for pw in pending if pw[4] in wait_classes]
    print(f"[kernel] injecting {len(pending)} waits, classes={sorted(wait_classes)}", flush=True)
    _inject_waits(nc, pending)
    _validate_pool_order(nc, pool_insts)
    nc.compile()
    return nc


def _validate_pool_order(nc, pool_insts):
    """The SWDGE ring is a FIFO: trigger_dma fires the oldest untriggered
    preps in Pool-stream order. Verify the scheduled Pool stream keeps each
    group of 7 preps contiguous and immediately followed by its trigger, and
    that group order is step-monotone per layer (release-wait liveness)."""
    seq = []
    for b in nc.m.functions[0].blocks:
        for i in b.instructions:
            if i.name in pool_insts:
                seq.append(pool_insts[i.name])
    groups = []
    i = 0
    while i < len(seq):
        kind, l, t = seq[i]
        assert kind == "prep", f"pool order: expected prep at {i}, got {seq[i]}"
        for j in range(7):
            assert seq[i + j] == ("prep", l, t), f"pool order: interleaved preps at {i+j}: {seq[i+j]} vs {(l, t)}"
        assert seq[i + 7] == ("trig", l, t), f"pool order: trigger mismatch at {i+7}: {seq[i+7]} vs {(l, t)}"
        groups.append((l, t))
        i += 8
    last_t = {0: -1, 1: -1}
    for l, t in groups:
        assert t > last_t[l], f"pool order: layer {l} trigger steps not monotone: {t} after {last_t[l]}"
        last_t[l] = t


def prep_inputs(x, h0, c0, W_ih, W_hh, b_ih, b_hh, T):
    """Host-side sharding/layout prep. Returns list of per-core input dicts."""
    TB = T * B
    xTf = np.ascontiguousarray(x.reshape(TB, IN).T)       # [IN, TB]
    bsum = b_ih + b_hh                                     # [L, 4H]
    ident = np.eye(B, dtype=np.float32)
    in_maps = []
    for c in range(N):
        sl = slice(c * HL, (c + 1) * HL)
        # local gate rows in i,f,o,g order
        rows = np.concatenate([
            np.arange(0 * H + c * HL, 0 * H + (c + 1) * HL),
            np.arange(1 * H + c * HL, 1 * H + (c + 1) * HL),
            np.arange(3 * H + c * HL, 3 * H + (c + 1) * HL),
            np.arange(2 * H + c * HL, 2 * H + (c + 1) * HL),
        ])
        xor_perm = np.concatenate([
            np.arange((c ^ j) * 128, ((c ^ j) + 1) * 128) for j in range(N)])
        m = {"xT": xTf, "ident": ident, "ones": np.ones((1, 128), np.float32)}
        for l in range(L):
            wiT = np.ascontiguousarray(W_ih[l][rows, :].T)   # [IN, 512]
            whT = np.ascontiguousarray(W_hh[l][rows, :].T)   # [H, 512]
            if l == 1:
                wiT = np.ascontiguousarray(wiT[xor_perm, :])
            whT = np.ascontiguousarray(whT[xor_perm, :])
            m[f"wih{l}"] = wiT
            m[f"whh{l}"] = whT
        m["bias"] = np.ascontiguousarray(bsum[:, rows])      # [L, 512]
        h0ag = np.zeros((L, 128, N * B), np.float32)
        for l in range(L):
            for j in range(N):
                cc = c ^ j
                h0ag[l][:, j * B:(j + 1) * B] = h0[:, l, cc * HL:(cc + 1) * HL].T
        m["h0ag"] = h0ag
        m["c0loc"] = np.ascontiguousarray(
            np.stack([c0[:, l, sl] for l in range(L)]))      # [L, B, HL]
        in_maps.append({k: np.ascontiguousarray(v, dtype=np.float32)
                        for k, v in m.items()})
    return in_maps


_NC_CACHE = {}
_SAN_CACHE = {}


def _build_sanitizer():
    """Tiny NEFF that zeroes the three cross-core semaphores on every core.
    Run before each main execution so static wait thresholds count from 0.
    No cross-core traffic, so it cannot race."""
    nc = bacc.Bacc(None, target_bir_lowering=False, debug=False)
    rs1 = nc.alloc_semaphore("rs1")
    rs2 = nc.alloc_semaphore("rs2")
    ls = nc.alloc_semaphore("lsend")
    src_d = nc.dram_tensor("s_in", [1, 4], F32, kind="ExternalInput")
    out_d = nc.dram_tensor("s_out", [1, 4], F32, kind="ExternalOutput")
    with tile.TileContext(nc) as tc:
        with tc.tile_pool(name="p", bufs=1) as pool:
            t = pool.tile([1, 4], F32)
            nc.gpsimd.sem_clear(rs1)
            nc.gpsimd.sem_clear(rs2)
            nc.gpsimd.sem_clear(ls)
            nc.sync.dma_start(t[:], src_d[:])
            nc.sync.dma_start(out_d[:], t[:])
    nc.compile()
    return nc


def sanitize():
    if "nc" not in _SAN_CACHE:
        _SAN_CACHE["nc"] = _build_sanitizer()
    z = np.zeros((1, 4), np.float32)
    run_bass_kernel_spmd(_SAN_CACHE["nc"], [{"s_in": z} for _ in range(N)],
                         core_ids=list(range(N)))


def run(inputs, T, trace=False):
    global LAST_EXEC_NS
    import time as _time
    if T not in _NC_CACHE:
        t0 = _time.monotonic()
        _NC_CACHE[T] = build(T)
        print(f"[kernel] build+schedule: {_time.monotonic()-t0:.1f}s", flush=True)
    nc = _NC_CACHE[T]
    in_maps = prep_inputs(
        inputs["x"], inputs["h0"], inputs["c0"], inputs["W_ih"], inputs["W_hh"],
        inputs["b_ih"], inputs["b_hh"], T)
    sanitize()
    t1 = _time.monotonic()
    res = run_bass_kernel_spmd(nc, in_maps, core_ids=list(range(N)), trace=trace)
    print(f"[kernel] compile+run: {_time.monotonic()-t1:.1f}s", flush=True)
    LAST_EXEC_NS = res.exec_time_ns
    y = np.concatenate([res.results[c]["y"] for c in range(N)], axis=2)   # [T,B,H]
    hn = np.concatenate([res.results[c]["hn"] for c in range(N)], axis=2) # [L,B,H]
    cn = np.concatenate([res.results[c]["cn"] for c in range(N)], axis=2)
    hn = np.ascontiguousarray(hn.transpose(1, 0, 2))                      # [B,L,H]
    cn = np.ascontiguousarray(cn.transpose(1, 0, 2))
    return y, (hn, cn)


def kernel(x, h0, c0, W_ih, W_hh, b_ih, b_hh):
    import os
    inputs = dict(x=np.asarray(x, np.float32), h0=np.asarray(h0, np.float32),
                  c0=np.asarray(c0, np.float32), W_ih=np.asarray(W_ih, np.float32),
                  W_hh=np.asarray(W_hh, np.float32), b_ih=np.asarray(b_ih, np.float32),
                  b_hh=np.asarray(b_hh, np.float32))
    trace = bool(int(os.environ.get("KERNEL_TRACE", "0")))
    return run(inputs, inputs["x"].shape[0], trace=trace)


def time_exec(inputs, T, iters=8):
    """Wall-clock the NEFF execution with device-resident inputs (no NTFF hook
    in this container). Returns (min_ns, avg_ns) over iters timed calls."""
    import time
    import jax
    import numpy as np
    from jax.sharding import Mesh, PartitionSpec, NamedSharding
    from jax.experimental.shard_map import shard_map
    from concourse import bass2jax

    if T not in _NC_CACHE:
        _NC_CACHE[T] = build(T)
    nc = _NC_CACHE[T]
    in_maps = prep_inputs(
        inputs["x"], inputs["h0"], inputs["c0"], inputs["W_ih"], inputs["W_hh"],
        inputs["b_ih"], inputs["b_hh"], T)

    import concourse.mybir as mb
    partition_name = nc.partition_id_tensor.name if nc.partition_id_tensor else None
    in_names, out_names, out_avals, zero_outs = [], [], [], []
    for alloc in nc.m.functions[0].allocations:
        if not isinstance(alloc, mb.MemoryLocationSet):
            continue
        name = alloc.memorylocations[0].name
        if alloc.kind == "ExternalInput":
            if name != partition_name:
                in_names.append(name)
        elif alloc.kind == "ExternalOutput":
            out_names.append(name)
            shape = tuple(alloc.tensor_shape)
            dtype = mb.dt.np(alloc.dtype)
            out_avals.append(jax.core.ShapedArray(shape, dtype))
            zero_outs.append(np.zeros(shape, dtype))
    n_params = len(in_names)
    n_outs = len(out_avals)
    all_names = list(in_names) + out_names
    if partition_name is not None:
        all_names.append(partition_name)
    donate = tuple(range(n_params, n_params + n_outs))

    def _body(*args):
        operands = list(args)
        if partition_name is not None:
            operands.append(bass2jax.partition_id_tensor())
        outs = bass2jax._bass_exec_p.bind(
            *operands, out_avals=tuple(out_avals), in_names=tuple(all_names),
            out_names=tuple(out_names), lowering_input_output_aliases=(),
            sim_require_finite=True, sim_require_nnan=True, nc=nc)
        return tuple(outs)

    devices = jax.devices()[:N]
    mesh = Mesh(np.asarray(devices), ("core",))
    in_specs = (PartitionSpec("core"),) * (n_params + n_outs)
    out_specs = (PartitionSpec("core"),) * len(out_names)
    fn = jax.jit(shard_map(_body, mesh=mesh, in_specs=in_specs,
                           out_specs=out_specs, check_rep=False),
                 donate_argnums=donate, keep_unused=True)
    sh = NamedSharding(mesh, PartitionSpec("core"))
    concat_in = [
        jax.device_put(np.concatenate([np.asarray(in_maps[c][n]) for c in range(N)], axis=0), sh)
        for n in in_names
    ]
    def fresh_zeros():
        return [jax.device_put(np.zeros((N * z.shape[0], *z.shape[1:]), z.dtype), sh)
                for z in zero_outs]

    # warmup (compile + load)
    sanitize()
    outs = fn(*concat_in, *fresh_zeros())
    jax.block_until_ready(outs)
    times = []
    for _ in range(iters):
        sanitize()
        zs = fresh_zeros()
        jax.block_until_ready(zs)
        t0 = time.perf_counter()
        outs = fn(*concat_in, *zs)
        jax.block_until_ready(outs)
        times.append(time.perf_counter() - t0)
    ns = [t * 1e9 for t in times]
    return min(ns), sum(ns) / len(ns)
